# revision 9
# baseline (speedup 1.0000x reference)
"""Multi-head attention (B=2, S=2048, D=1024, H=16, dk=dv=64) on 8 TRN2 NeuronCores.

Sharding: data-parallel over batch (2 groups of 4 cores), tensor-parallel over
heads within a group (4 heads per core). Mask replicated per batch group.
Each core computes: q/k/v projections for its 4 heads, masked softmax
attention (written out in full), attention @ v, and a partial output
projection (its heads' slice of Wo). Host concatenates attention outputs and
sums the 4 partial output projections per batch.

Matmuls run in fp32r (TF32-like: fp32 layout, round-to-nearest-even at 11
mantissa bits) at 4x the fp32 PE rate.
"""
import numpy as np

B, S, D = 2, 2048, 1024
H, DK, DV = 16, 64, 64
N_CORES = 8
CPB = 4            # cores per batch group
HPC = H // CPB     # heads per core = 4
HD = HPC * DK      # head dims per core = 256
P = 128
NQ = S // P        # 16 q-chunks
NKC = S // P       # 16 k-chunks
FC = D // P        # 8 feature chunks

_nc_cache = None


def _build_program():
    from contextlib import ExitStack
    import concourse.tile as tile
    from concourse import bacc, mybir

    f32 = mybir.dt.float32
    f32r = mybir.dt.float32r
    i8 = mybir.dt.int8
    Exp = mybir.ActivationFunctionType.Exp
    Ident = mybir.ActivationFunctionType.Identity
    mult = mybir.AluOpType.mult

    nc = bacc.Bacc("TRN2", target_bir_lowering=False, debug=False,
                   num_devices=N_CORES)

    xqT = nc.dram_tensor("xqT", [D, S], f32r, kind="ExternalInput").ap()
    xkT = nc.dram_tensor("xkT", [D, S], f32r, kind="ExternalInput").ap()
    xvT = nc.dram_tensor("xvT", [D, S], f32r, kind="ExternalInput").ap()
    m8 = nc.dram_tensor("m8", [S, S], i8, kind="ExternalInput").ap()
    wq = nc.dram_tensor("wq", [D, HD], f32r, kind="ExternalInput").ap()
    wk = nc.dram_tensor("wk", [D, HD], f32r, kind="ExternalInput").ap()
    wv = nc.dram_tensor("wv", [D, HD], f32r, kind="ExternalInput").ap()
    wo = nc.dram_tensor("wo", [HD, D], f32r, kind="ExternalInput").ap()
    bqT = nc.dram_tensor("bqT", [P, HD // P], f32, kind="ExternalInput").ap()
    bkT = nc.dram_tensor("bkT", [P, HD // P], f32, kind="ExternalInput").ap()
    bv = nc.dram_tensor("bv", [1, HD], f32r, kind="ExternalInput").ap()
    bo4 = nc.dram_tensor("bo4", [1, D], f32r, kind="ExternalInput").ap()

    attn = nc.dram_tensor("attn", [HPC, S, S], f32, kind="ExternalOutput").ap()
    outp = nc.dram_tensor("outp", [S, D], f32, kind="ExternalOutput").ap()

    from concourse.masks import make_identity

    with tile.TileContext(nc) as tc, ExitStack() as ctx:
        const = ctx.enter_context(tc.tile_pool(name="const", bufs=1))
        persist = ctx.enter_context(tc.tile_pool(name="persist", bufs=1))

        ident = const.tile([P, P], f32, tag="ident")
        make_identity(nc, ident[:])
        ones_f = const.tile([1, P], f32, tag="ones_f")
        nc.vector.memset(ones_f[:], 1.0)
        ones = const.tile([1, P], f32r, tag="ones")
        nc.vector.tensor_copy(ones[:], ones_f[:])
        bq_sb = const.tile([P, HD // P], f32, tag="bq_sb")
        nc.sync.dma_start(bq_sb[:], bqT[:])
        bk_sb = const.tile([P, HD // P], f32, tag="bk_sb")
        nc.sync.dma_start(bk_sb[:], bkT[:])
        bv_sb = const.tile([1, HD], f32r, tag="bv_sb")
        nc.sync.dma_start(bv_sb[:], bv[:])
        bo4_sb = const.tile([1, D], f32r, tag="bo4_sb")
        nc.sync.dma_start(bo4_sb[:], bo4[:])

        # projection results: qT/kT [dk-part, chunk, tok], v [tok-part, kc, hd]
        qT_r = persist.tile([P, HD // P, S], f32r, tag="qT_r")
        kT_r = persist.tile([P, HD // P, S], f32r, tag="kT_r")
        v_r = persist.tile([P, NKC, HD], f32r, tag="v_r")
        avT_r = persist.tile([64, HPC, S], f32r, tag="avT_r")

        # ---- Phase A: projections ----
        for name, xT, w_dram, b_sb, isv in (
            ("q", xqT, wq, bq_sb, False),
            ("k", xkT, wk, bk_sb, False),
            ("v", xvT, wv, None, True),
        ):
            with ExitStack() as pctx:
                xpool = pctx.enter_context(
                    tc.tile_pool(name=f"xp_{name}", bufs=FC + 1))
                wpool = pctx.enter_context(
                    tc.tile_pool(name=f"wp_{name}", bufs=1))
                pp = pctx.enter_context(
                    tc.tile_pool(name=f"pp_{name}", bufs=2, space="PSUM"))
                w_sb = wpool.tile([P, FC, HD], f32r, tag="w")
                nc.sync.dma_start(w_sb[:],
                                  w_dram.rearrange("(c p) n -> p c n", p=P))
                xs = []
                for fc in range(FC):
                    xt = xpool.tile([P, S], f32r, tag="x")
                    nc.sync.dma_start(xt[:], xT[fc * P:(fc + 1) * P, :])
                    xs.append(xt)
                if not isv:
                    dst = qT_r if name == "q" else kT_r
                    for m in range(HD // P):
                        for tq in range(S // 512):
                            ps = pp.tile([P, 512], f32, tag="ps")
                            for fc in range(FC):
                                nc.tensor.matmul(
                                    ps[:],
                                    w_sb[:, fc, m * P:(m + 1) * P],
                                    xs[fc][:, tq * 512:(tq + 1) * 512],
                                    start=(fc == 0), stop=(fc == FC - 1))
                            nc.scalar.activation(
                                dst[:, m, tq * 512:(tq + 1) * 512], ps[:],
                                Ident, bias=b_sb[:, m:m + 1], scale=1.0)
                else:
                    for t in range(NKC):
                        ps = pp.tile([P, HD], f32, tag="psv")
                        for fc in range(FC):
                            nc.tensor.matmul(
                                ps[:],
                                xs[fc][:, t * P:(t + 1) * P],
                                w_sb[:, fc, :],
                                start=(fc == 0), stop=False)
                        nc.tensor.matmul(ps[:], ones[:], bv_sb[:],
                                         start=False, stop=True)
                        nc.vector.tensor_copy(v_r[:, t, :], ps[:])

        # ---- Phase B: attention ----
        with ExitStack() as actx:
            mpool = actx.enter_context(tc.tile_pool(name="mpool", bufs=3))
            m8pool = actx.enter_context(tc.tile_pool(name="m8pool", bufs=2))
            prpool = actx.enter_context(tc.tile_pool(name="prpool", bufs=2))
            pmpool = actx.enter_context(tc.tile_pool(name="pmpool", bufs=2))
            pnpool = actx.enter_context(tc.tile_pool(name="pnpool", bufs=2))
            rpool = actx.enter_context(tc.tile_pool(name="rpool", bufs=8))
            ptpool = actx.enter_context(tc.tile_pool(name="ptpool", bufs=2))
            psE = actx.enter_context(
                tc.tile_pool(name="psE", bufs=1, space="PSUM"))
            psT = actx.enter_context(
                tc.tile_pool(name="psT", bufs=2, space="PSUM"))
            psAV = actx.enter_context(
                tc.tile_pool(name="psAV", bufs=2, space="PSUM"))

            for qs in range(NQ // 2):
                maskf = []
                for sub in range(2):
                    qi = 2 * qs + sub
                    m8t = m8pool.tile([P, S], i8, tag="m8t")
                    nc.sync.dma_start(m8t[:], m8[qi * P:(qi + 1) * P, :])
                    mf = mpool.tile([P, S], f32, tag="maskf")
                    nc.vector.tensor_copy(mf[:], m8t[:])
                    maskf.append(mf)

                for h in range(HPC):
                    avp = psAV.tile([64, 256], f32, tag="avp")
                    if True:
                        hp0 = 64 * (h % 2)
                        hc = h // 2
                        pt = ptpool.tile([P, NKC, 256], f32r, tag="pt")
                        for sub in range(2):
                            qi = 2 * qs + sub
                            E = psE.tile([P, S], f32, tag="E")
                            for n in range(S // 512):
                                nc.tensor.matmul(
                                    E[:, n * 512:(n + 1) * 512],
                                    qT_r[hp0:hp0 + 64, hc, qi * P:(qi + 1) * P],
                                    kT_r[hp0:hp0 + 64, hc, n * 512:(n + 1) * 512],
                                    start=True, stop=True)
                            p_raw = prpool.tile([P, S], f32, tag="p_raw")
                            nc.scalar.activation(p_raw[:], E[:], Exp,
                                                 bias=0.0, scale=0.125)
                            p_m = pmpool.tile([P, S], f32, tag="p_m")
                            rs = rpool.tile([P, 1], f32, tag="rs")
                            nc.vector.scalar_tensor_tensor(
                                out=p_m[:], in0=p_raw[:], scalar=1.0,
                                in1=maskf[sub][:], op0=mult, op1=mult,
                                accum_out=rs[:])
                            rc = rpool.tile([P, 1], f32, tag="rc")
                            nc.vector.reciprocal(rc[:], rs[:])
                            p_n = pnpool.tile([P, S], f32, tag="p_n")
                            nc.gpsimd.tensor_scalar_mul(p_n[:], p_m[:], rc[:])
                            nc.sync.dma_start(
                                attn[h, qi * P:(qi + 1) * P, :], p_n[:])
                            for g in range(4):
                                Tp = psT.tile([P, 512], f32, tag="Tp")
                                for j in range(4):
                                    blk = 4 * g + j
                                    nc.tensor.transpose(
                                        Tp[:, j * P:(j + 1) * P],
                                        p_n[:, blk * P:(blk + 1) * P],
                                        ident[:])
                                dst = pt[:, 4 * g:4 * g + 4,
                                         sub * P:(sub + 1) * P]
                                src = Tp[:].rearrange("p (a b) -> p a b", a=4)
                                if g < 2:
                                    nc.scalar.copy(dst, src)
                                else:
                                    nc.vector.tensor_copy(dst, src)
                        for kc in range(NKC):
                            nc.tensor.matmul(
                                avp[:],
                                v_r[:, kc, h * 64:h * 64 + 64],
                                pt[:, kc, :],
                                start=(kc == 0), stop=(kc == NKC - 1))
                    nc.scalar.copy(
                        avT_r[:, h, qs * 256:(qs + 1) * 256], avp[:])

        # ---- Phase C: output projection ----
        with ExitStack() as octx:
            psO = octx.enter_context(
                tc.tile_pool(name="psO", bufs=2, space="PSUM"))
            opool = octx.enter_context(tc.tile_pool(name="opool", bufs=2))
            wopool = octx.enter_context(tc.tile_pool(name="wopool", bufs=1))
            wo_sb = wopool.tile([64, HPC, D], f32r, tag="wo")
            nc.sync.dma_start(wo_sb[:], wo.rearrange("(c p) n -> p c n", p=64))
            for t in range(NQ):
                O = psO.tile([P, D], f32, tag="O")
                for n2 in range(D // 512):
                    for dc in range(HPC):
                        nc.tensor.matmul(
                            O[:, n2 * 512:(n2 + 1) * 512],
                            avT_r[:, dc, t * P:(t + 1) * P],
                            wo_sb[:, dc, n2 * 512:(n2 + 1) * 512],
                            start=(dc == 0), stop=False)
                    nc.tensor.matmul(
                        O[:, n2 * 512:(n2 + 1) * 512],
                        ones[:], bo4_sb[:, n2 * 512:(n2 + 1) * 512],
                        start=False, stop=True)
                osb = opool.tile([P, D], f32, tag="osb")
                nc.scalar.copy(osb[:], O[:])
                nc.sync.dma_start(outp[t * P:(t + 1) * P, :], osb[:])

    nc.compile()
    return nc


def kernel(query, key, value, mask, Wq, bq, Wk, bk, Wv, bv, Wo, bo,
           trace=False):
    global _nc_cache
    from concourse.bass_utils import run_bass_kernel_spmd

    if _nc_cache is None:
        _nc_cache = _build_program()
    nc = _nc_cache

    query = np.asarray(query, np.float32)
    key = np.asarray(key, np.float32)
    value = np.asarray(value, np.float32)
    mask = np.asarray(mask)
    Wq, bq = np.asarray(Wq, np.float32), np.asarray(bq, np.float32)
    Wk, bk = np.asarray(Wk, np.float32), np.asarray(bk, np.float32)
    Wv, bvv = np.asarray(Wv, np.float32), np.asarray(bv, np.float32)
    Wo, bo = np.asarray(Wo, np.float32), np.asarray(bo, np.float32)

    # host-side shard/layout prep
    xqT = [np.ascontiguousarray(query[b].T) for b in range(B)]
    xkT = [np.ascontiguousarray(key[b].T) for b in range(B)]
    xvT = [np.ascontiguousarray(value[b].T) for b in range(B)]
    m8 = [np.ascontiguousarray(mask[b].astype(np.int8)) for b in range(B)]

    in_maps = []
    for c in range(N_CORES):
        b = c // CPB
        h0 = (c % CPB) * HPC          # first head on this core
        lo, hi = h0 * DK, h0 * DK + HD
        in_maps.append({
            "xqT": xqT[b], "xkT": xkT[b], "xvT": xvT[b], "m8": m8[b],
            "wq": np.ascontiguousarray(Wq[:, lo:hi]),
            "wk": np.ascontiguousarray(Wk[:, lo:hi]),
            "wv": np.ascontiguousarray(Wv[:, lo:hi]),
            "wo": np.ascontiguousarray(Wo[lo:hi, :]),
            "bqT": np.ascontiguousarray(bq[lo:hi].reshape(HD // P, P).T),
            "bkT": np.ascontiguousarray(bk[lo:hi].reshape(HD // P, P).T),
            "bv": np.ascontiguousarray(bvv[lo:hi].reshape(1, HD)),
            "bo4": np.ascontiguousarray((bo * 0.25).reshape(1, D)),
        })

    res = run_bass_kernel_spmd(nc, in_maps, core_ids=list(range(N_CORES)),
                               trace=trace)

    attention = np.empty((B, H, S, S), np.float32)
    out = np.zeros((B, S, D), np.float32)
    for c in range(N_CORES):
        b = c // CPB
        h0 = (c % CPB) * HPC
        attention[b, h0:h0 + HPC] = res.results[c]["attn"]
        out[b] += res.results[c]["outp"]

    if trace:
        kernel.last_exec_time_ns = res.exec_time_ns
        kernel.last_results = res
    return (out, attention)


# revision 11
# speedup vs baseline: 3.7975x; 3.7975x over previous
"""Multi-head attention (B=2, S=2048, D=1024, H=16, dk=dv=64) on 8 TRN2 NeuronCores.

Sharding: data-parallel over batch (2 groups of 4 cores), tensor-parallel over
heads within a group (4 heads per core). Mask replicated per batch group.
Each core computes: q/k/v projections for its 4 heads, masked softmax
attention (written out in full), attention @ v, and a partial output
projection (its heads' slice of Wo). Host concatenates attention outputs and
sums the 4 partial output projections per batch.

Matmuls run in fp32r (TF32-like: fp32 layout, round-to-nearest-even at 11
mantissa bits) at 4x the fp32 PE rate.
"""
import numpy as np

B, S, D = 2, 2048, 1024
H, DK, DV = 16, 64, 64
N_CORES = 8
CPB = 4            # cores per batch group
HPC = H // CPB     # heads per core = 4
HD = HPC * DK      # head dims per core = 256
P = 128
NQ = S // P        # 16 q-chunks
NKC = S // P       # 16 k-chunks
FC = D // P        # 8 feature chunks

_nc_cache = None


def _build_program():
    from contextlib import ExitStack
    import concourse.tile as tile
    from concourse import bacc, mybir

    f32 = mybir.dt.float32
    f32r = mybir.dt.float32r
    i8 = mybir.dt.int8
    Exp = mybir.ActivationFunctionType.Exp
    Ident = mybir.ActivationFunctionType.Identity
    mult = mybir.AluOpType.mult

    nc = bacc.Bacc("TRN2", target_bir_lowering=False, debug=False,
                   num_devices=N_CORES)

    xqT = nc.dram_tensor("xqT", [D, S], f32r, kind="ExternalInput").ap()
    xkT = nc.dram_tensor("xkT", [D, S], f32r, kind="ExternalInput").ap()
    xvT = nc.dram_tensor("xvT", [D, S], f32r, kind="ExternalInput").ap()
    m8 = nc.dram_tensor("m8", [S, S], i8, kind="ExternalInput").ap()
    wq = nc.dram_tensor("wq", [D, HD], f32r, kind="ExternalInput").ap()
    wk = nc.dram_tensor("wk", [D, HD], f32r, kind="ExternalInput").ap()
    wv = nc.dram_tensor("wv", [D, HD], f32r, kind="ExternalInput").ap()
    wo = nc.dram_tensor("wo", [HD, D], f32r, kind="ExternalInput").ap()
    bqT = nc.dram_tensor("bqT", [P, HD // P], f32, kind="ExternalInput").ap()
    bkT = nc.dram_tensor("bkT", [P, HD // P], f32, kind="ExternalInput").ap()
    bv = nc.dram_tensor("bv", [1, HD], f32r, kind="ExternalInput").ap()
    bo4 = nc.dram_tensor("bo4", [1, D], f32r, kind="ExternalInput").ap()

    attn = nc.dram_tensor("attn", [HPC, S, S], f32, kind="ExternalOutput").ap()
    outp = nc.dram_tensor("outp", [S, D], f32, kind="ExternalOutput").ap()

    from concourse.masks import make_identity

    with tile.TileContext(nc) as tc, ExitStack() as ctx:
        const = ctx.enter_context(tc.tile_pool(name="const", bufs=1))
        persist = ctx.enter_context(tc.tile_pool(name="persist", bufs=1))

        ident = const.tile([P, P], f32, tag="ident")
        make_identity(nc, ident[:])
        ones_f = const.tile([1, P], f32, tag="ones_f")
        nc.vector.memset(ones_f[:], 1.0)
        ones = const.tile([1, P], f32r, tag="ones")
        nc.vector.tensor_copy(ones[:], ones_f[:])
        bq_sb = const.tile([P, HD // P], f32, tag="bq_sb")
        nc.sync.dma_start(bq_sb[:], bqT[:])
        bk_sb = const.tile([P, HD // P], f32, tag="bk_sb")
        nc.sync.dma_start(bk_sb[:], bkT[:])
        bv_sb = const.tile([1, HD], f32r, tag="bv_sb")
        nc.sync.dma_start(bv_sb[:], bv[:])
        bo4_sb = const.tile([1, D], f32r, tag="bo4_sb")
        nc.sync.dma_start(bo4_sb[:], bo4[:])

        # projection results: qT/kT [dk-part, chunk, tok], v [tok-part, kc, hd]
        qT_r = persist.tile([P, HD // P, S], f32r, tag="qT_r")
        kT_r = persist.tile([P, HD // P, S], f32r, tag="kT_r")
        v_r = persist.tile([P, NKC, HD], f32r, tag="v_r")
        avT_r = persist.tile([64, HPC, S], f32r, tag="avT_r")

        # ---- Phase A: projections ----
        for name, xT, w_dram, b_sb, isv in (
            ("q", xqT, wq, bq_sb, False),
            ("k", xkT, wk, bk_sb, False),
            ("v", xvT, wv, None, True),
        ):
            with ExitStack() as pctx:
                xpool = pctx.enter_context(
                    tc.tile_pool(name=f"xp_{name}", bufs=FC + 1))
                wpool = pctx.enter_context(
                    tc.tile_pool(name=f"wp_{name}", bufs=1))
                pp = pctx.enter_context(
                    tc.tile_pool(name=f"pp_{name}", bufs=2, space="PSUM"))
                w_sb = wpool.tile([P, FC, HD], f32r, tag="w")
                nc.sync.dma_start(w_sb[:],
                                  w_dram.rearrange("(c p) n -> p c n", p=P))
                xs = []
                for fc in range(FC):
                    xt = xpool.tile([P, S], f32r, tag="x")
                    nc.sync.dma_start(xt[:], xT[fc * P:(fc + 1) * P, :])
                    xs.append(xt)
                if not isv:
                    dst = qT_r if name == "q" else kT_r
                    for m in range(HD // P):
                        for tq in range(S // 512):
                            ps = pp.tile([P, 512], f32, tag="ps")
                            for fc in range(FC):
                                nc.tensor.matmul(
                                    ps[:],
                                    w_sb[:, fc, m * P:(m + 1) * P],
                                    xs[fc][:, tq * 512:(tq + 1) * 512],
                                    start=(fc == 0), stop=(fc == FC - 1))
                            nc.scalar.activation(
                                dst[:, m, tq * 512:(tq + 1) * 512], ps[:],
                                Ident, bias=b_sb[:, m:m + 1], scale=1.0)
                else:
                    for t in range(NKC):
                        ps = pp.tile([P, HD], f32, tag="psv")
                        for fc in range(FC):
                            nc.tensor.matmul(
                                ps[:],
                                xs[fc][:, t * P:(t + 1) * P],
                                w_sb[:, fc, :],
                                start=(fc == 0), stop=False)
                        nc.tensor.matmul(ps[:], ones[:], bv_sb[:],
                                         start=False, stop=True)
                        nc.vector.tensor_copy(v_r[:, t, :], ps[:])

        # ---- Phase B: attention ----
        with ExitStack() as actx:
            mpool = actx.enter_context(tc.tile_pool(name="mpool", bufs=3))
            m8pool = actx.enter_context(tc.tile_pool(name="m8pool", bufs=2))
            prpool = actx.enter_context(tc.tile_pool(name="prpool", bufs=2))
            pmpool = actx.enter_context(tc.tile_pool(name="pmpool", bufs=2))
            pnpool = actx.enter_context(tc.tile_pool(name="pnpool", bufs=2))
            rpool = actx.enter_context(tc.tile_pool(name="rpool", bufs=8))
            ptpool = actx.enter_context(tc.tile_pool(name="ptpool", bufs=2))
            psE = actx.enter_context(
                tc.tile_pool(name="psE", bufs=1, space="PSUM"))
            psT = actx.enter_context(
                tc.tile_pool(name="psT", bufs=2, space="PSUM"))
            psAV = actx.enter_context(
                tc.tile_pool(name="psAV", bufs=2, space="PSUM"))

            sub_ = mybir.AluOpType.subtract
            for qs in range(NQ // 2):
                maskf = []
                for sub in range(2):
                    qi = 2 * qs + sub
                    m8t = m8pool.tile([P, S], i8, tag="m8t")
                    nc.sync.dma_start(m8t[:], m8[qi * P:(qi + 1) * P, :])
                    # mask bias: (m-1)*80000 -> 0 for keep, -80000 for drop
                    # (exp scale 0.125 turns it into -10000)
                    mf = mpool.tile([P, S], f32, tag="maskf")
                    nc.vector.tensor_scalar(
                        out=mf[:], in0=m8t[:], scalar1=1, scalar2=80000.0,
                        op0=sub_, op1=mult)
                    maskf.append(mf)

                for h in range(HPC):
                    avp = psAV.tile([64, 256], f32, tag="avp")
                    if True:
                        hp0 = 64 * (h % 2)
                        hc = h // 2
                        pt = ptpool.tile([P, NKC, 256], f32r, tag="pt")
                        for sub in range(2):
                            qi = 2 * qs + sub
                            E = psE.tile([P, S], f32, tag="E")
                            for n in range(S // 512):
                                nc.tensor.matmul(
                                    E[:, n * 512:(n + 1) * 512],
                                    qT_r[hp0:hp0 + 64, hc, qi * P:(qi + 1) * P],
                                    kT_r[hp0:hp0 + 64, hc, n * 512:(n + 1) * 512],
                                    start=True, stop=True)
                            p_in = pmpool.tile([P, S], f32, tag="p_in")
                            nc.vector.tensor_add(p_in[:], E[:], maskf[sub][:])
                            p_raw = prpool.tile([P, S], f32, tag="p_raw")
                            rs = rpool.tile([P, 1], f32, tag="rs")
                            nc.scalar.activation(p_raw[:], p_in[:], Exp,
                                                 bias=0.0, scale=0.125,
                                                 accum_out=rs[:])
                            rc = rpool.tile([P, 1], f32, tag="rc")
                            nc.vector.reciprocal(rc[:], rs[:])
                            p_n = pnpool.tile([P, S], f32, tag="p_n")
                            nc.vector.tensor_scalar_mul(p_n[:], p_raw[:], rc[:])
                            nc.sync.dma_start(
                                attn[h, qi * P:(qi + 1) * P, :], p_n[:])
                            for g in range(4):
                                Tp = psT.tile([P, 512], f32, tag="Tp")
                                for j in range(4):
                                    blk = 4 * g + j
                                    nc.tensor.transpose(
                                        Tp[:, j * P:(j + 1) * P],
                                        p_n[:, blk * P:(blk + 1) * P],
                                        ident[:])
                                dst = pt[:, 4 * g:4 * g + 4,
                                         sub * P:(sub + 1) * P]
                                src = Tp[:].rearrange("p (a b) -> p a b", a=4)
                                if g < 2:
                                    nc.scalar.copy(dst, src)
                                else:
                                    nc.vector.tensor_copy(dst, src)
                        for kc in range(NKC):
                            nc.tensor.matmul(
                                avp[:],
                                v_r[:, kc, h * 64:h * 64 + 64],
                                pt[:, kc, :],
                                start=(kc == 0), stop=(kc == NKC - 1))
                    nc.scalar.copy(
                        avT_r[:, h, qs * 256:(qs + 1) * 256], avp[:])

        # ---- Phase C: output projection ----
        with ExitStack() as octx:
            psO = octx.enter_context(
                tc.tile_pool(name="psO", bufs=2, space="PSUM"))
            opool = octx.enter_context(tc.tile_pool(name="opool", bufs=2))
            wopool = octx.enter_context(tc.tile_pool(name="wopool", bufs=1))
            wo_sb = wopool.tile([64, HPC, D], f32r, tag="wo")
            nc.sync.dma_start(wo_sb[:], wo.rearrange("(c p) n -> p c n", p=64))
            for t in range(NQ):
                O = psO.tile([P, D], f32, tag="O")
                for n2 in range(D // 512):
                    for dc in range(HPC):
                        nc.tensor.matmul(
                            O[:, n2 * 512:(n2 + 1) * 512],
                            avT_r[:, dc, t * P:(t + 1) * P],
                            wo_sb[:, dc, n2 * 512:(n2 + 1) * 512],
                            start=(dc == 0), stop=False)
                    nc.tensor.matmul(
                        O[:, n2 * 512:(n2 + 1) * 512],
                        ones[:], bo4_sb[:, n2 * 512:(n2 + 1) * 512],
                        start=False, stop=True)
                osb = opool.tile([P, D], f32, tag="osb")
                nc.scalar.copy(osb[:], O[:])
                nc.sync.dma_start(outp[t * P:(t + 1) * P, :], osb[:])

    nc.compile()
    return nc


def kernel(query, key, value, mask, Wq, bq, Wk, bk, Wv, bv, Wo, bo,
           trace=False):
    global _nc_cache
    from concourse.bass_utils import run_bass_kernel_spmd

    if _nc_cache is None:
        _nc_cache = _build_program()
    nc = _nc_cache

    query = np.asarray(query, np.float32)
    key = np.asarray(key, np.float32)
    value = np.asarray(value, np.float32)
    mask = np.asarray(mask)
    Wq, bq = np.asarray(Wq, np.float32), np.asarray(bq, np.float32)
    Wk, bk = np.asarray(Wk, np.float32), np.asarray(bk, np.float32)
    Wv, bvv = np.asarray(Wv, np.float32), np.asarray(bv, np.float32)
    Wo, bo = np.asarray(Wo, np.float32), np.asarray(bo, np.float32)

    # host-side shard/layout prep
    xqT = [np.ascontiguousarray(query[b].T) for b in range(B)]
    xkT = [np.ascontiguousarray(key[b].T) for b in range(B)]
    xvT = [np.ascontiguousarray(value[b].T) for b in range(B)]
    m8 = [np.ascontiguousarray(mask[b].astype(np.int8)) for b in range(B)]

    in_maps = []
    for c in range(N_CORES):
        b = c // CPB
        h0 = (c % CPB) * HPC          # first head on this core
        lo, hi = h0 * DK, h0 * DK + HD
        in_maps.append({
            "xqT": xqT[b], "xkT": xkT[b], "xvT": xvT[b], "m8": m8[b],
            "wq": np.ascontiguousarray(Wq[:, lo:hi]),
            "wk": np.ascontiguousarray(Wk[:, lo:hi]),
            "wv": np.ascontiguousarray(Wv[:, lo:hi]),
            "wo": np.ascontiguousarray(Wo[lo:hi, :]),
            "bqT": np.ascontiguousarray(bq[lo:hi].reshape(HD // P, P).T),
            "bkT": np.ascontiguousarray(bk[lo:hi].reshape(HD // P, P).T),
            "bv": np.ascontiguousarray(bvv[lo:hi].reshape(1, HD)),
            "bo4": np.ascontiguousarray((bo * 0.25).reshape(1, D)),
        })

    res = run_bass_kernel_spmd(nc, in_maps, core_ids=list(range(N_CORES)),
                               trace=trace)

    attention = np.empty((B, H, S, S), np.float32)
    out = np.zeros((B, S, D), np.float32)
    for c in range(N_CORES):
        b = c // CPB
        h0 = (c % CPB) * HPC
        attention[b, h0:h0 + HPC] = res.results[c]["attn"]
        out[b] += res.results[c]["outp"]

    if trace:
        kernel.last_exec_time_ns = res.exec_time_ns
        kernel.last_results = res
    return (out, attention)


# revision 15
# speedup vs baseline: 3.8497x; 1.0137x over previous
"""Multi-head attention (B=2, S=2048, D=1024, H=16, dk=dv=64) on 8 TRN2 NeuronCores.

Sharding: data-parallel over batch (2 groups of 4 cores), tensor-parallel over
heads within a group (4 heads per core). Mask replicated per batch group.
Each core computes: q/k/v projections for its 4 heads, masked softmax
attention (written out in full), attention @ v, and a partial output
projection (its heads' slice of Wo). Host concatenates attention outputs and
sums the 4 partial output projections per batch.

Matmuls run in fp32r (TF32-like: fp32 layout, round-to-nearest-even at 11
mantissa bits) at 4x the fp32 PE rate.
"""
import numpy as np

B, S, D = 2, 2048, 1024
H, DK, DV = 16, 64, 64
N_CORES = 8
CPB = 4            # cores per batch group
HPC = H // CPB     # heads per core = 4
HD = HPC * DK      # head dims per core = 256
P = 128
NQ = S // P        # 16 q-chunks
NKC = S // P       # 16 k-chunks
FC = D // P        # 8 feature chunks

_nc_cache = None


def _build_program():
    from contextlib import ExitStack
    import concourse.tile as tile
    from concourse import bacc, mybir

    f32 = mybir.dt.float32
    f32r = mybir.dt.float32r
    i8 = mybir.dt.int8
    Exp = mybir.ActivationFunctionType.Exp
    Ident = mybir.ActivationFunctionType.Identity
    mult = mybir.AluOpType.mult

    nc = bacc.Bacc("TRN2", target_bir_lowering=False, debug=False,
                   num_devices=N_CORES)

    xqT = nc.dram_tensor("xqT", [D, S], f32r, kind="ExternalInput").ap()
    xkT = nc.dram_tensor("xkT", [D, S], f32r, kind="ExternalInput").ap()
    xvT = nc.dram_tensor("xvT", [D, S], f32r, kind="ExternalInput").ap()
    m8 = nc.dram_tensor("m8", [S, S], i8, kind="ExternalInput").ap()
    wq = nc.dram_tensor("wq", [D, HD], f32r, kind="ExternalInput").ap()
    wk = nc.dram_tensor("wk", [D, HD], f32r, kind="ExternalInput").ap()
    wv = nc.dram_tensor("wv", [D, HD], f32r, kind="ExternalInput").ap()
    wo = nc.dram_tensor("wo", [HD, D], f32r, kind="ExternalInput").ap()
    bqT = nc.dram_tensor("bqT", [P, HD // P], f32, kind="ExternalInput").ap()
    bkT = nc.dram_tensor("bkT", [P, HD // P], f32, kind="ExternalInput").ap()
    bv = nc.dram_tensor("bv", [1, HD], f32r, kind="ExternalInput").ap()
    bo4 = nc.dram_tensor("bo4", [1, D], f32r, kind="ExternalInput").ap()

    attn = nc.dram_tensor("attn", [HPC, S, S], f32, kind="ExternalOutput").ap()
    outp = nc.dram_tensor("outp", [S, D], f32, kind="ExternalOutput").ap()

    from concourse.masks import make_identity

    with tile.TileContext(nc) as tc, ExitStack() as ctx:
        const = ctx.enter_context(tc.tile_pool(name="const", bufs=1))
        persist = ctx.enter_context(tc.tile_pool(name="persist", bufs=1))

        ident = const.tile([P, P], f32, tag="ident")
        make_identity(nc, ident[:])
        ones_f = const.tile([1, P], f32, tag="ones_f")
        nc.vector.memset(ones_f[:], 1.0)
        ones = const.tile([1, P], f32r, tag="ones")
        nc.vector.tensor_copy(ones[:], ones_f[:])
        bq_sb = const.tile([P, HD // P], f32, tag="bq_sb")
        nc.sync.dma_start(bq_sb[:], bqT[:])
        bk_sb = const.tile([P, HD // P], f32, tag="bk_sb")
        nc.sync.dma_start(bk_sb[:], bkT[:])
        bv_sb = const.tile([1, HD], f32r, tag="bv_sb")
        nc.sync.dma_start(bv_sb[:], bv[:])
        bo4_sb = const.tile([1, D], f32r, tag="bo4_sb")
        nc.sync.dma_start(bo4_sb[:], bo4[:])

        # projection results: qT/kT [dk-part, chunk, tok], v [tok-part, kc, hd]
        qT_r = persist.tile([P, HD // P, S], f32r, tag="qT_r")
        kT_r = persist.tile([P, HD // P, S], f32r, tag="kT_r")
        v_r = persist.tile([P, NKC, HD], f32r, tag="v_r")
        avT_r = persist.tile([64, HPC, S], f32r, tag="avT_r")

        # ---- Phase A: projections ----
        with ExitStack() as prctx:
            xpool = prctx.enter_context(
                tc.tile_pool(name="xp", bufs=FC + 4))
            wpool = prctx.enter_context(tc.tile_pool(name="wp", bufs=2))
            pp = prctx.enter_context(
                tc.tile_pool(name="pp", bufs=2, space="PSUM"))
            for name, xT, w_dram, b_sb, isv in (
                ("q", xqT, wq, bq_sb, False),
                ("k", xkT, wk, bk_sb, False),
                ("v", xvT, wv, None, True),
            ):
                w_sb = wpool.tile([P, FC, HD], f32r, tag="w")
                nc.sync.dma_start(w_sb[:],
                                  w_dram.rearrange("(c p) n -> p c n", p=P))
                xs = []
                for fc in range(FC):
                    xt = xpool.tile([P, S], f32r, tag="x")
                    nc.sync.dma_start(xt[:], xT[fc * P:(fc + 1) * P, :])
                    xs.append(xt)
                if not isv:
                    dst = qT_r if name == "q" else kT_r
                    for m in range(HD // P):
                        for tq in range(S // 512):
                            ps = pp.tile([P, 512], f32, tag="ps")
                            for fc in range(FC):
                                nc.tensor.matmul(
                                    ps[:],
                                    w_sb[:, fc, m * P:(m + 1) * P],
                                    xs[fc][:, tq * 512:(tq + 1) * 512],
                                    start=(fc == 0), stop=(fc == FC - 1))
                            nc.scalar.activation(
                                dst[:, m, tq * 512:(tq + 1) * 512], ps[:],
                                Ident, bias=b_sb[:, m:m + 1], scale=1.0)
                else:
                    for t in range(NKC):
                        ps = pp.tile([P, HD], f32, tag="psv")
                        for fc in range(FC):
                            nc.tensor.matmul(
                                ps[:],
                                xs[fc][:, t * P:(t + 1) * P],
                                w_sb[:, fc, :],
                                start=(fc == 0), stop=False)
                        nc.tensor.matmul(ps[:], ones[:], bv_sb[:],
                                         start=False, stop=True)
                        nc.vector.tensor_copy(v_r[:, t, :], ps[:])

        # ---- Phase B: attention ----
        with ExitStack() as actx:
            mpool = actx.enter_context(tc.tile_pool(name="mpool", bufs=3))
            m8pool = actx.enter_context(tc.tile_pool(name="m8pool", bufs=2))
            prpool = actx.enter_context(tc.tile_pool(name="prpool", bufs=3))
            pmpool = actx.enter_context(tc.tile_pool(name="pmpool", bufs=4))
            pnpool = actx.enter_context(tc.tile_pool(name="pnpool", bufs=2))
            rpool = actx.enter_context(tc.tile_pool(name="rpool", bufs=8))
            ptpool = actx.enter_context(tc.tile_pool(name="ptpool", bufs=2))
            psE = actx.enter_context(
                tc.tile_pool(name="psE", bufs=2, space="PSUM"))
            psT = actx.enter_context(
                tc.tile_pool(name="psT", bufs=2, space="PSUM"))
            psAV = actx.enter_context(
                tc.tile_pool(name="psAV", bufs=2, space="PSUM"))

            sub_ = mybir.AluOpType.subtract
            for qs in range(NQ // 2):
                maskf = []
                for sub in range(2):
                    qi = 2 * qs + sub
                    m8t = m8pool.tile([P, S], i8, tag="m8t")
                    nc.sync.dma_start(m8t[:], m8[qi * P:(qi + 1) * P, :])
                    # mask bias: (m-1)*80000 -> 0 for keep, -80000 for drop
                    # (exp scale 0.125 turns it into -10000)
                    mf = mpool.tile([P, S], f32, tag="maskf")
                    nc.vector.tensor_scalar(
                        out=mf[:], in0=m8t[:], scalar1=1, scalar2=80000.0,
                        op0=sub_, op1=mult)
                    maskf.append(mf)

                for h in range(HPC):
                    avp = psAV.tile([64, 256], f32, tag="avp")
                    if True:
                        hp0 = 64 * (h % 2)
                        hc = h // 2
                        pt = ptpool.tile([P, NKC, 256], f32r, tag="pt")
                        for sub in range(2):
                            qi = 2 * qs + sub
                            p_raw = prpool.tile([P, S], f32, tag="p_raw")
                            rs = rpool.tile([P, 2], f32, tag="rs")
                            for half in range(2):
                                E = psE.tile([P, S // 2], f32, tag="E")
                                for n in range(2):
                                    c0 = half * 1024 + n * 512
                                    nc.tensor.matmul(
                                        E[:, n * 512:(n + 1) * 512],
                                        qT_r[hp0:hp0 + 64, hc,
                                             qi * P:(qi + 1) * P],
                                        kT_r[hp0:hp0 + 64, hc, c0:c0 + 512],
                                        start=True, stop=True)
                                p_in = pmpool.tile([P, S // 2], f32,
                                                   tag="p_in")
                                nc.vector.tensor_add(
                                    p_in[:], E[:],
                                    maskf[sub][:, half * 1024:
                                               (half + 1) * 1024])
                                nc.scalar.activation(
                                    p_raw[:, half * 1024:(half + 1) * 1024],
                                    p_in[:], Exp, bias=0.0, scale=0.125,
                                    accum_out=rs[:, half:half + 1])
                            rst = rpool.tile([P, 1], f32, tag="rst")
                            nc.vector.tensor_add(rst[:], rs[:, 0:1],
                                                 rs[:, 1:2])
                            rc = rpool.tile([P, 1], f32, tag="rc")
                            nc.vector.reciprocal(rc[:], rst[:])
                            p_n = pnpool.tile([P, S], f32, tag="p_n")
                            nc.vector.tensor_scalar_mul(p_n[:], p_raw[:], rc[:])
                            nc.sync.dma_start(
                                attn[h, qi * P:(qi + 1) * P, :], p_n[:])
                            for g in range(4):
                                Tp = psT.tile([P, 512], f32, tag="Tp")
                                for j in range(4):
                                    blk = 4 * g + j
                                    nc.tensor.transpose(
                                        Tp[:, j * P:(j + 1) * P],
                                        p_n[:, blk * P:(blk + 1) * P],
                                        ident[:])
                                dst = pt[:, 4 * g:4 * g + 4,
                                         sub * P:(sub + 1) * P]
                                src = Tp[:].rearrange("p (a b) -> p a b", a=4)
                                if g < 2:
                                    nc.scalar.copy(dst, src)
                                else:
                                    nc.vector.tensor_copy(dst, src)
                        for kc in range(NKC):
                            nc.tensor.matmul(
                                avp[:],
                                v_r[:, kc, h * 64:h * 64 + 64],
                                pt[:, kc, :],
                                start=(kc == 0), stop=(kc == NKC - 1))
                    nc.scalar.copy(
                        avT_r[:, h, qs * 256:(qs + 1) * 256], avp[:])

        # ---- Phase C: output projection ----
        with ExitStack() as octx:
            psO = octx.enter_context(
                tc.tile_pool(name="psO", bufs=2, space="PSUM"))
            opool = octx.enter_context(tc.tile_pool(name="opool", bufs=2))
            wopool = octx.enter_context(tc.tile_pool(name="wopool", bufs=1))
            wo_sb = wopool.tile([64, HPC, D], f32r, tag="wo")
            nc.sync.dma_start(wo_sb[:], wo.rearrange("(c p) n -> p c n", p=64))
            for t in range(NQ):
                O = psO.tile([P, D], f32, tag="O")
                for n2 in range(D // 512):
                    for dc in range(HPC):
                        nc.tensor.matmul(
                            O[:, n2 * 512:(n2 + 1) * 512],
                            avT_r[:, dc, t * P:(t + 1) * P],
                            wo_sb[:, dc, n2 * 512:(n2 + 1) * 512],
                            start=(dc == 0), stop=False)
                    nc.tensor.matmul(
                        O[:, n2 * 512:(n2 + 1) * 512],
                        ones[:], bo4_sb[:, n2 * 512:(n2 + 1) * 512],
                        start=False, stop=True)
                osb = opool.tile([P, D], f32, tag="osb")
                nc.scalar.copy(osb[:], O[:])
                nc.sync.dma_start(outp[t * P:(t + 1) * P, :], osb[:])

    nc.compile()
    return nc


def kernel(query, key, value, mask, Wq, bq, Wk, bk, Wv, bv, Wo, bo,
           trace=False):
    global _nc_cache
    from concourse.bass_utils import run_bass_kernel_spmd

    if _nc_cache is None:
        _nc_cache = _build_program()
    nc = _nc_cache

    query = np.asarray(query, np.float32)
    key = np.asarray(key, np.float32)
    value = np.asarray(value, np.float32)
    mask = np.asarray(mask)
    Wq, bq = np.asarray(Wq, np.float32), np.asarray(bq, np.float32)
    Wk, bk = np.asarray(Wk, np.float32), np.asarray(bk, np.float32)
    Wv, bvv = np.asarray(Wv, np.float32), np.asarray(bv, np.float32)
    Wo, bo = np.asarray(Wo, np.float32), np.asarray(bo, np.float32)

    # host-side shard/layout prep
    xqT = [np.ascontiguousarray(query[b].T) for b in range(B)]
    xkT = [np.ascontiguousarray(key[b].T) for b in range(B)]
    xvT = [np.ascontiguousarray(value[b].T) for b in range(B)]
    m8 = [np.ascontiguousarray(mask[b].astype(np.int8)) for b in range(B)]

    in_maps = []
    for c in range(N_CORES):
        b = c // CPB
        h0 = (c % CPB) * HPC          # first head on this core
        lo, hi = h0 * DK, h0 * DK + HD
        in_maps.append({
            "xqT": xqT[b], "xkT": xkT[b], "xvT": xvT[b], "m8": m8[b],
            "wq": np.ascontiguousarray(Wq[:, lo:hi]),
            "wk": np.ascontiguousarray(Wk[:, lo:hi]),
            "wv": np.ascontiguousarray(Wv[:, lo:hi]),
            "wo": np.ascontiguousarray(Wo[lo:hi, :]),
            "bqT": np.ascontiguousarray(bq[lo:hi].reshape(HD // P, P).T),
            "bkT": np.ascontiguousarray(bk[lo:hi].reshape(HD // P, P).T),
            "bv": np.ascontiguousarray(bvv[lo:hi].reshape(1, HD)),
            "bo4": np.ascontiguousarray((bo * 0.25).reshape(1, D)),
        })

    res = run_bass_kernel_spmd(nc, in_maps, core_ids=list(range(N_CORES)),
                               trace=trace)

    attention = np.empty((B, H, S, S), np.float32)
    out = np.zeros((B, S, D), np.float32)
    for c in range(N_CORES):
        b = c // CPB
        h0 = (c % CPB) * HPC
        attention[b, h0:h0 + HPC] = res.results[c]["attn"]
        out[b] += res.results[c]["outp"]

    if trace:
        kernel.last_exec_time_ns = res.exec_time_ns
        kernel.last_results = res
    return (out, attention)


# revision 17
# speedup vs baseline: 4.2318x; 1.0992x over previous
"""Multi-head attention (B=2, S=2048, D=1024, H=16, dk=dv=64) on 8 TRN2 NeuronCores.

Sharding: data-parallel over batch (2 groups of 4 cores), tensor-parallel over
heads within a group (4 heads per core). Mask replicated per batch group.
Each core computes: q/k/v projections for its 4 heads, masked softmax
attention (written out in full), attention @ v, and a partial output
projection (its heads' slice of Wo). Host concatenates attention outputs and
sums the 4 partial output projections per batch.

Matmuls run in fp32r (TF32-like: fp32 layout, round-to-nearest-even at 11
mantissa bits) at 4x the fp32 PE rate.
"""
import numpy as np

B, S, D = 2, 2048, 1024
H, DK, DV = 16, 64, 64
N_CORES = 8
CPB = 4            # cores per batch group
HPC = H // CPB     # heads per core = 4
HD = HPC * DK      # head dims per core = 256
P = 128
NQ = S // P        # 16 q-chunks
NKC = S // P       # 16 k-chunks
FC = D // P        # 8 feature chunks

_nc_cache = None


def _build_program():
    from contextlib import ExitStack
    import concourse.tile as tile
    from concourse import bacc, mybir

    f32 = mybir.dt.float32
    f32r = mybir.dt.float32r
    i8 = mybir.dt.int8
    Exp = mybir.ActivationFunctionType.Exp
    Ident = mybir.ActivationFunctionType.Identity
    mult = mybir.AluOpType.mult

    nc = bacc.Bacc("TRN2", target_bir_lowering=False, debug=False,
                   num_devices=N_CORES)

    xqT = nc.dram_tensor("xqT", [D, S], f32r, kind="ExternalInput").ap()
    xkT = nc.dram_tensor("xkT", [D, S], f32r, kind="ExternalInput").ap()
    xvT = nc.dram_tensor("xvT", [D, S], f32r, kind="ExternalInput").ap()
    m8 = nc.dram_tensor("m8", [S, S], i8, kind="ExternalInput").ap()
    wq = nc.dram_tensor("wq", [D, HD], f32r, kind="ExternalInput").ap()
    wk = nc.dram_tensor("wk", [D, HD], f32r, kind="ExternalInput").ap()
    wv = nc.dram_tensor("wv", [D, HD], f32r, kind="ExternalInput").ap()
    wo = nc.dram_tensor("wo", [HD, D], f32r, kind="ExternalInput").ap()
    bqT = nc.dram_tensor("bqT", [P, HD // P], f32, kind="ExternalInput").ap()
    bkT = nc.dram_tensor("bkT", [P, HD // P], f32, kind="ExternalInput").ap()
    bv = nc.dram_tensor("bv", [1, HD], f32r, kind="ExternalInput").ap()
    bo4 = nc.dram_tensor("bo4", [1, D], f32r, kind="ExternalInput").ap()

    attn = nc.dram_tensor("attn", [HPC, S, S], f32, kind="ExternalOutput").ap()
    outp = nc.dram_tensor("outp", [S, D], f32, kind="ExternalOutput").ap()

    from concourse.masks import make_identity

    with tile.TileContext(nc) as tc, ExitStack() as ctx:
        const = ctx.enter_context(tc.tile_pool(name="const", bufs=1))
        persist = ctx.enter_context(tc.tile_pool(name="persist", bufs=1))

        ident = const.tile([P, P], f32, tag="ident")
        make_identity(nc, ident[:])
        ones_f = const.tile([1, P], f32, tag="ones_f")
        nc.vector.memset(ones_f[:], 1.0)
        ones = const.tile([1, P], f32r, tag="ones")
        nc.vector.tensor_copy(ones[:], ones_f[:])
        bq_sb = const.tile([P, HD // P], f32, tag="bq_sb")
        nc.sync.dma_start(bq_sb[:], bqT[:])
        bk_sb = const.tile([P, HD // P], f32, tag="bk_sb")
        nc.sync.dma_start(bk_sb[:], bkT[:])
        bv_sb = const.tile([1, HD], f32r, tag="bv_sb")
        nc.sync.dma_start(bv_sb[:], bv[:])
        bo4_sb = const.tile([1, D], f32r, tag="bo4_sb")
        nc.sync.dma_start(bo4_sb[:], bo4[:])

        # projection results: qT/kT [dk-part, chunk, tok], v [tok-part, kc, hd]
        qT_r = persist.tile([P, HD // P, S], f32r, tag="qT_r")
        kT_r = persist.tile([P, HD // P, S], f32r, tag="kT_r")
        v_r = persist.tile([P, NKC, HD], f32r, tag="v_r")
        avT_r = persist.tile([64, HPC, S], f32r, tag="avT_r")

        # ---- Phase A: projections ----
        with ExitStack() as prctx:
            xpool = prctx.enter_context(
                tc.tile_pool(name="xp", bufs=FC + 4))
            wpool = prctx.enter_context(tc.tile_pool(name="wp", bufs=2))
            pp = prctx.enter_context(
                tc.tile_pool(name="pp", bufs=2, space="PSUM"))
            for name, xT, w_dram, b_sb, isv in (
                ("q", xqT, wq, bq_sb, False),
                ("k", xkT, wk, bk_sb, False),
                ("v", xvT, wv, None, True),
            ):
                w_sb = wpool.tile([P, FC, HD], f32r, tag="w")
                nc.sync.dma_start(w_sb[:],
                                  w_dram.rearrange("(c p) n -> p c n", p=P))
                xs = []
                for fc in range(FC):
                    xt = xpool.tile([P, S], f32r, tag="x")
                    nc.sync.dma_start(xt[:], xT[fc * P:(fc + 1) * P, :])
                    xs.append(xt)
                if not isv:
                    dst = qT_r if name == "q" else kT_r
                    for m in range(HD // P):
                        for tq in range(S // 512):
                            ps = pp.tile([P, 512], f32, tag="ps")
                            for fc in range(FC):
                                nc.tensor.matmul(
                                    ps[:],
                                    w_sb[:, fc, m * P:(m + 1) * P],
                                    xs[fc][:, tq * 512:(tq + 1) * 512],
                                    start=(fc == 0), stop=(fc == FC - 1))
                            nc.scalar.activation(
                                dst[:, m, tq * 512:(tq + 1) * 512], ps[:],
                                Ident, bias=b_sb[:, m:m + 1], scale=1.0)
                else:
                    for t in range(NKC):
                        ps = pp.tile([P, HD], f32, tag="psv")
                        for fc in range(FC):
                            nc.tensor.matmul(
                                ps[:],
                                xs[fc][:, t * P:(t + 1) * P],
                                w_sb[:, fc, :],
                                start=(fc == 0), stop=False)
                        nc.tensor.matmul(ps[:], ones[:], bv_sb[:],
                                         start=False, stop=True)
                        nc.vector.tensor_copy(v_r[:, t, :], ps[:])

        # ---- Phase B: attention ----
        with ExitStack() as actx:
            mpool = actx.enter_context(tc.tile_pool(name="mpool", bufs=3))
            m8pool = actx.enter_context(tc.tile_pool(name="m8pool", bufs=2))
            prpool = actx.enter_context(tc.tile_pool(name="prpool", bufs=3))
            pmpool = actx.enter_context(tc.tile_pool(name="pmpool", bufs=4))
            pnpool = actx.enter_context(tc.tile_pool(name="pnpool", bufs=2))
            rpool = actx.enter_context(tc.tile_pool(name="rpool", bufs=8))
            ptpool = actx.enter_context(tc.tile_pool(name="ptpool", bufs=2))
            psE = actx.enter_context(
                tc.tile_pool(name="psE", bufs=2, space="PSUM"))
            psT = actx.enter_context(
                tc.tile_pool(name="psT", bufs=3, space="PSUM"))
            psAV = actx.enter_context(
                tc.tile_pool(name="psAV", bufs=1, space="PSUM"))

            sub_ = mybir.AluOpType.subtract
            for qs in range(NQ // 2):
                maskf = []
                for sub in range(2):
                    qi = 2 * qs + sub
                    m8t = m8pool.tile([P, S], i8, tag="m8t")
                    nc.sync.dma_start(m8t[:], m8[qi * P:(qi + 1) * P, :])
                    # mask bias: (m-1)*80000 -> 0 for keep, -80000 for drop
                    # (exp scale 0.125 turns it into -10000)
                    mf = mpool.tile([P, S], f32, tag="maskf")
                    nc.vector.tensor_scalar(
                        out=mf[:], in0=m8t[:], scalar1=1, scalar2=80000.0,
                        op0=sub_, op1=mult)
                    maskf.append(mf)

                for h in range(HPC):
                    avp = psAV.tile([64, 256], f32, tag="avp")
                    if True:
                        hp0 = 64 * (h % 2)
                        hc = h // 2
                        pt = ptpool.tile([P, NKC, 256], f32r, tag="pt")
                        for sub in range(2):
                            qi = 2 * qs + sub
                            p_raw = prpool.tile([P, S], f32, tag="p_raw")
                            rs = rpool.tile([P, 2], f32, tag="rs")
                            for half in range(2):
                                E = psE.tile([P, S // 2], f32, tag="E")
                                for n in range(2):
                                    c0 = half * 1024 + n * 512
                                    nc.tensor.matmul(
                                        E[:, n * 512:(n + 1) * 512],
                                        qT_r[hp0:hp0 + 64, hc,
                                             qi * P:(qi + 1) * P],
                                        kT_r[hp0:hp0 + 64, hc, c0:c0 + 512],
                                        start=True, stop=True)
                                p_in = pmpool.tile([P, S // 2], f32,
                                                   tag="p_in")
                                nc.vector.tensor_add(
                                    p_in[:], E[:],
                                    maskf[sub][:, half * 1024:
                                               (half + 1) * 1024])
                                nc.scalar.activation(
                                    p_raw[:, half * 1024:(half + 1) * 1024],
                                    p_in[:], Exp, bias=0.0, scale=0.125,
                                    accum_out=rs[:, half:half + 1])
                            rst = rpool.tile([P, 1], f32, tag="rst")
                            nc.vector.tensor_add(rst[:], rs[:, 0:1],
                                                 rs[:, 1:2])
                            rc = rpool.tile([P, 1], f32, tag="rc")
                            nc.vector.reciprocal(rc[:], rst[:])
                            p_n = pnpool.tile([P, S], f32, tag="p_n")
                            nc.vector.tensor_scalar_mul(p_n[:], p_raw[:], rc[:])
                            nc.sync.dma_start(
                                attn[h, qi * P:(qi + 1) * P, :], p_n[:])
                            for g in range(4):
                                Tp = psT.tile([P, 512], f32, tag="Tp")
                                for j in range(4):
                                    blk = 4 * g + j
                                    nc.tensor.transpose(
                                        Tp[:, j * P:(j + 1) * P],
                                        p_n[:, blk * P:(blk + 1) * P],
                                        ident[:])
                                dst = pt[:, 4 * g:4 * g + 4,
                                         sub * P:(sub + 1) * P]
                                src = Tp[:].rearrange("p (a b) -> p a b", a=4)
                                if g < 2:
                                    nc.scalar.copy(dst, src)
                                else:
                                    nc.vector.tensor_copy(dst, src)
                        for kc in range(NKC):
                            nc.tensor.matmul(
                                avp[:],
                                v_r[:, kc, h * 64:h * 64 + 64],
                                pt[:, kc, :],
                                start=(kc == 0), stop=(kc == NKC - 1))
                    nc.scalar.copy(
                        avT_r[:, h, qs * 256:(qs + 1) * 256], avp[:])

        # ---- Phase C: output projection ----
        with ExitStack() as octx:
            psO = octx.enter_context(
                tc.tile_pool(name="psO", bufs=2, space="PSUM"))
            opool = octx.enter_context(tc.tile_pool(name="opool", bufs=2))
            wopool = octx.enter_context(tc.tile_pool(name="wopool", bufs=1))
            wo_sb = wopool.tile([64, HPC, D], f32r, tag="wo")
            nc.sync.dma_start(wo_sb[:], wo.rearrange("(c p) n -> p c n", p=64))
            for t in range(NQ):
                O = psO.tile([P, D], f32, tag="O")
                for n2 in range(D // 512):
                    for dc in range(HPC):
                        nc.tensor.matmul(
                            O[:, n2 * 512:(n2 + 1) * 512],
                            avT_r[:, dc, t * P:(t + 1) * P],
                            wo_sb[:, dc, n2 * 512:(n2 + 1) * 512],
                            start=(dc == 0), stop=False)
                    nc.tensor.matmul(
                        O[:, n2 * 512:(n2 + 1) * 512],
                        ones[:], bo4_sb[:, n2 * 512:(n2 + 1) * 512],
                        start=False, stop=True)
                osb = opool.tile([P, D], f32, tag="osb")
                nc.scalar.copy(osb[:], O[:])
                nc.sync.dma_start(outp[t * P:(t + 1) * P, :], osb[:])

    nc.compile()
    return nc


def kernel(query, key, value, mask, Wq, bq, Wk, bk, Wv, bv, Wo, bo,
           trace=False):
    global _nc_cache
    from concourse.bass_utils import run_bass_kernel_spmd

    if _nc_cache is None:
        _nc_cache = _build_program()
    nc = _nc_cache

    query = np.asarray(query, np.float32)
    key = np.asarray(key, np.float32)
    value = np.asarray(value, np.float32)
    mask = np.asarray(mask)
    Wq, bq = np.asarray(Wq, np.float32), np.asarray(bq, np.float32)
    Wk, bk = np.asarray(Wk, np.float32), np.asarray(bk, np.float32)
    Wv, bvv = np.asarray(Wv, np.float32), np.asarray(bv, np.float32)
    Wo, bo = np.asarray(Wo, np.float32), np.asarray(bo, np.float32)

    # host-side shard/layout prep
    xqT = [np.ascontiguousarray(query[b].T) for b in range(B)]
    xkT = [np.ascontiguousarray(key[b].T) for b in range(B)]
    xvT = [np.ascontiguousarray(value[b].T) for b in range(B)]
    m8 = [np.ascontiguousarray(mask[b].astype(np.int8)) for b in range(B)]

    in_maps = []
    for c in range(N_CORES):
        b = c // CPB
        h0 = (c % CPB) * HPC          # first head on this core
        lo, hi = h0 * DK, h0 * DK + HD
        in_maps.append({
            "xqT": xqT[b], "xkT": xkT[b], "xvT": xvT[b], "m8": m8[b],
            "wq": np.ascontiguousarray(Wq[:, lo:hi]),
            "wk": np.ascontiguousarray(Wk[:, lo:hi]),
            "wv": np.ascontiguousarray(Wv[:, lo:hi]),
            "wo": np.ascontiguousarray(Wo[lo:hi, :]),
            "bqT": np.ascontiguousarray(bq[lo:hi].reshape(HD // P, P).T),
            "bkT": np.ascontiguousarray(bk[lo:hi].reshape(HD // P, P).T),
            "bv": np.ascontiguousarray(bvv[lo:hi].reshape(1, HD)),
            "bo4": np.ascontiguousarray((bo * 0.25).reshape(1, D)),
        })

    res = run_bass_kernel_spmd(nc, in_maps, core_ids=list(range(N_CORES)),
                               trace=trace)

    attention = np.empty((B, H, S, S), np.float32)
    out = np.zeros((B, S, D), np.float32)
    for c in range(N_CORES):
        b = c // CPB
        h0 = (c % CPB) * HPC
        attention[b, h0:h0 + HPC] = res.results[c]["attn"]
        out[b] += res.results[c]["outp"]

    if trace:
        kernel.last_exec_time_ns = res.exec_time_ns
        kernel.last_results = res
    return (out, attention)


# revision 27
# speedup vs baseline: 4.3212x; 1.0211x over previous
"""Multi-head attention (B=2, S=2048, D=1024, H=16, dk=dv=64) on 8 TRN2 NeuronCores.

Sharding: data-parallel over batch (2 groups of 4 cores), tensor-parallel over
heads within a group (4 heads per core). Mask replicated per batch group.
Each core computes: q/k/v projections for its 4 heads, masked softmax
attention (written out in full), attention @ v, and a partial output
projection (its heads' slice of Wo). Host concatenates attention outputs and
sums the 4 partial output projections per batch.

Matmuls run in fp32r (TF32-like: fp32 layout, round-to-nearest-even at 11
mantissa bits) at 4x the fp32 PE rate.
"""
import numpy as np

B, S, D = 2, 2048, 1024
H, DK, DV = 16, 64, 64
N_CORES = 8
CPB = 4            # cores per batch group
HPC = H // CPB     # heads per core = 4
HD = HPC * DK      # head dims per core = 256
P = 128
NQ = S // P        # 16 q-chunks
NKC = S // P       # 16 k-chunks
FC = D // P        # 8 feature chunks

_nc_cache = None


def _build_program():
    from contextlib import ExitStack
    import concourse.tile as tile
    from concourse import bacc, mybir

    f32 = mybir.dt.float32
    f32r = mybir.dt.float32r
    i8 = mybir.dt.int8
    Exp = mybir.ActivationFunctionType.Exp
    Ident = mybir.ActivationFunctionType.Identity
    mult = mybir.AluOpType.mult

    nc = bacc.Bacc("TRN2", target_bir_lowering=False, debug=False,
                   num_devices=N_CORES)

    xqT = nc.dram_tensor("xqT", [D, S], f32r, kind="ExternalInput").ap()
    xkT = nc.dram_tensor("xkT", [D, S], f32r, kind="ExternalInput").ap()
    xvT = nc.dram_tensor("xvT", [D, S], f32r, kind="ExternalInput").ap()
    m8 = nc.dram_tensor("m8", [S, S], i8, kind="ExternalInput").ap()
    wq = nc.dram_tensor("wq", [D, HD], f32r, kind="ExternalInput").ap()
    wk = nc.dram_tensor("wk", [D, HD], f32r, kind="ExternalInput").ap()
    wv = nc.dram_tensor("wv", [D, HD], f32r, kind="ExternalInput").ap()
    # wo rows augmented: per head-chunk, 64 Wo rows + 1 bias row (bo/16)
    wo = nc.dram_tensor("wo", [HPC * 65, D], f32r, kind="ExternalInput").ap()
    bqT = nc.dram_tensor("bqT", [P, HD // P], f32, kind="ExternalInput").ap()
    bkT = nc.dram_tensor("bkT", [P, HD // P], f32, kind="ExternalInput").ap()
    bv = nc.dram_tensor("bv", [1, HD], f32r, kind="ExternalInput").ap()

    attn = nc.dram_tensor("attn", [HPC, S, S], f32, kind="ExternalOutput").ap()
    outp = nc.dram_tensor("outp", [S, D], f32, kind="ExternalOutput").ap()

    from concourse.masks import make_identity

    with tile.TileContext(nc) as tc, ExitStack() as ctx:
        const = ctx.enter_context(tc.tile_pool(name="const", bufs=1))
        persist = ctx.enter_context(tc.tile_pool(name="persist", bufs=1))

        ident = const.tile([P, P], f32, tag="ident")
        make_identity(nc, ident[:])
        ones_f = const.tile([1, P], f32, tag="ones_f")
        nc.vector.memset(ones_f[:], 1.0)
        ones = const.tile([1, P], f32r, tag="ones")
        nc.vector.tensor_copy(ones[:], ones_f[:])
        bq_sb = const.tile([P, HD // P], f32, tag="bq_sb")
        nc.sync.dma_start(bq_sb[:], bqT[:])
        bk_sb = const.tile([P, HD // P], f32, tag="bk_sb")
        nc.sync.dma_start(bk_sb[:], bkT[:])
        bv_sb = const.tile([1, HD], f32r, tag="bv_sb")
        nc.sync.dma_start(bv_sb[:], bv[:])


        # projection results: qT/kT [dk-part, chunk, tok], v [tok-part, kc, hd]
        qT_r = persist.tile([P, HD // P, S], f32r, tag="qT_r")
        kT_r = persist.tile([P, HD // P, S], f32r, tag="kT_r")
        v_r = persist.tile([P, NKC, HD], f32r, tag="v_r")
        avT_r = persist.tile([65, HPC, S], f32r, tag="avT_r")
        nc.vector.memset(avT_r[64:65, :, :].bitcast(f32), 1.0)

        # ---- Phase A: projections ----
        with ExitStack() as prctx:
            xpool = prctx.enter_context(
                tc.tile_pool(name="xp", bufs=FC + 4))
            wpool = prctx.enter_context(tc.tile_pool(name="wp", bufs=2))
            pp = prctx.enter_context(
                tc.tile_pool(name="pp", bufs=2, space="PSUM"))
            for name, xT, w_dram, b_sb, isv in (
                ("q", xqT, wq, bq_sb, False),
                ("k", xkT, wk, bk_sb, False),
                ("v", xvT, wv, None, True),
            ):
                w_sb = wpool.tile([P, FC, HD], f32r, tag="w")
                nc.sync.dma_start(w_sb[:],
                                  w_dram.rearrange("(c p) n -> p c n", p=P))
                xs = []
                for fc in range(FC):
                    xt = xpool.tile([P, S], f32r, tag="x")
                    nc.sync.dma_start(xt[:], xT[fc * P:(fc + 1) * P, :])
                    xs.append(xt)
                if not isv:
                    dst = qT_r if name == "q" else kT_r
                    for m in range(HD // P):
                        for tq in range(S // 512):
                            ps = pp.tile([P, 512], f32, tag="ps")
                            for fc in range(FC):
                                nc.tensor.matmul(
                                    ps[:],
                                    w_sb[:, fc, m * P:(m + 1) * P],
                                    xs[fc][:, tq * 512:(tq + 1) * 512],
                                    start=(fc == 0), stop=(fc == FC - 1))
                            nc.scalar.activation(
                                dst[:, m, tq * 512:(tq + 1) * 512], ps[:],
                                Ident, bias=b_sb[:, m:m + 1], scale=1.0)
                else:
                    for t in range(NKC):
                        ps = pp.tile([P, HD], f32, tag="psv")
                        for fc in range(FC):
                            nc.tensor.matmul(
                                ps[:],
                                xs[fc][:, t * P:(t + 1) * P],
                                w_sb[:, fc, :],
                                start=(fc == 0), stop=False)
                        nc.tensor.matmul(ps[:], ones[:], bv_sb[:],
                                         start=False, stop=True)
                        nc.vector.tensor_copy(v_r[:, t, :], ps[:])

        # ---- Phase B: attention ----
        with ExitStack() as actx:
            mpool = actx.enter_context(tc.tile_pool(name="mpool", bufs=2))
            m8pool = actx.enter_context(tc.tile_pool(name="m8pool", bufs=2))
            prpool = actx.enter_context(tc.tile_pool(name="prpool", bufs=2))
            pmpool = actx.enter_context(tc.tile_pool(name="pmpool", bufs=4))
            pnpool = actx.enter_context(tc.tile_pool(name="pnpool", bufs=2))
            rpool = actx.enter_context(tc.tile_pool(name="rpool", bufs=8))
            ptpool = actx.enter_context(tc.tile_pool(name="ptpool", bufs=2))
            psE = actx.enter_context(
                tc.tile_pool(name="psE", bufs=2, space="PSUM"))
            psT = actx.enter_context(
                tc.tile_pool(name="psT", bufs=3, space="PSUM"))
            psAV = actx.enter_context(
                tc.tile_pool(name="psAV", bufs=1, space="PSUM"))
            opool = actx.enter_context(tc.tile_pool(name="opool", bufs=3))
            wopool = actx.enter_context(tc.tile_pool(name="wopool", bufs=1))
            wo_sb = wopool.tile([65, HPC, D], f32r, tag="wo")
            nc.sync.dma_start(wo_sb[:],
                              wo.rearrange("(c p) n -> p c n", p=65))

            sub_ = mybir.AluOpType.subtract
            for qs in range(NQ // 2):
                maskf = []
                for sub in range(2):
                    qi = 2 * qs + sub
                    m8t = m8pool.tile([P, S], i8, tag="m8t")
                    nc.sync.dma_start(m8t[:], m8[qi * P:(qi + 1) * P, :])
                    # mask bias: (m-1)*80000 -> 0 for keep, -80000 for drop
                    # (exp scale 0.125 turns it into -10000)
                    mf = mpool.tile([P, S], f32, tag="maskf")
                    nc.vector.tensor_scalar(
                        out=mf[:], in0=m8t[:], scalar1=1, scalar2=80000.0,
                        op0=sub_, op1=mult)
                    maskf.append(mf)

                for h in range(HPC):
                    avp = psAV.tile([64, 256], f32, tag="avp")
                    if True:
                        hp0 = 64 * (h % 2)
                        hc = h // 2
                        pt = ptpool.tile([P, NKC, 256], f32r, tag="pt")
                        for sub in range(2):
                            qi = 2 * qs + sub
                            p_raw = prpool.tile([P, S], f32, tag="p_raw")
                            rs = rpool.tile([P, 2], f32, tag="rs")
                            for half in range(2):
                                E = psE.tile([P, S // 2], f32, tag="E")
                                for n in range(2):
                                    c0 = half * 1024 + n * 512
                                    nc.tensor.matmul(
                                        E[:, n * 512:(n + 1) * 512],
                                        qT_r[hp0:hp0 + 64, hc,
                                             qi * P:(qi + 1) * P],
                                        kT_r[hp0:hp0 + 64, hc, c0:c0 + 512],
                                        start=True, stop=True)
                                p_in = pmpool.tile([P, S // 2], f32,
                                                   tag="p_in")
                                nc.vector.tensor_add(
                                    p_in[:], E[:],
                                    maskf[sub][:, half * 1024:
                                               (half + 1) * 1024])
                                nc.scalar.activation(
                                    p_raw[:, half * 1024:(half + 1) * 1024],
                                    p_in[:], Exp, bias=0.0, scale=0.125,
                                    accum_out=rs[:, half:half + 1])
                            rst = rpool.tile([P, 1], f32, tag="rst")
                            nc.vector.tensor_add(rst[:], rs[:, 0:1],
                                                 rs[:, 1:2])
                            rc = rpool.tile([P, 1], f32, tag="rc")
                            nc.vector.reciprocal(rc[:], rst[:])
                            p_n = pnpool.tile([P, S], f32, tag="p_n")
                            nc.vector.tensor_scalar_mul(p_n[:], p_raw[:], rc[:])
                            nc.sync.dma_start(
                                attn[h, qi * P:(qi + 1) * P, :], p_n[:])
                            for g in range(4):
                                Tp = psT.tile([P, 512], f32, tag="Tp")
                                for j in range(4):
                                    blk = 4 * g + j
                                    nc.tensor.transpose(
                                        Tp[:, j * P:(j + 1) * P],
                                        p_n[:, blk * P:(blk + 1) * P],
                                        ident[:])
                                dst = pt[:, 4 * g:4 * g + 4,
                                         sub * P:(sub + 1) * P]
                                src = Tp[:].rearrange("p (a b) -> p a b", a=4)
                                if g < 2:
                                    nc.scalar.copy(dst, src)
                                else:
                                    nc.vector.tensor_copy(dst, src)
                        for kc in range(NKC):
                            nc.tensor.matmul(
                                avp[:],
                                v_r[:, kc, h * 64:h * 64 + 64],
                                pt[:, kc, :],
                                start=(kc == 0), stop=(kc == NKC - 1))
                    nc.scalar.copy(
                        avT_r[0:64, h, qs * 256:(qs + 1) * 256], avp[:])

                # output projection for the two finished token chunks
                for t in (2 * qs, 2 * qs + 1):
                    for n2 in range(D // 512):
                        O = psT.tile([P, 512], f32, tag="Tp")
                        for dc in range(HPC):
                            nc.tensor.matmul(
                                O[:],
                                avT_r[:, dc, t * P:(t + 1) * P],
                                wo_sb[:, dc, n2 * 512:(n2 + 1) * 512],
                                start=(dc == 0), stop=(dc == HPC - 1))
                        osb = opool.tile([P, 512], f32, tag="osb")
                        nc.scalar.copy(osb[:], O[:])
                        nc.sync.dma_start(
                            outp[t * P:(t + 1) * P,
                                 n2 * 512:(n2 + 1) * 512], osb[:])

    nc.compile()
    return nc


def kernel(query, key, value, mask, Wq, bq, Wk, bk, Wv, bv, Wo, bo,
           trace=False):
    global _nc_cache
    from concourse.bass_utils import run_bass_kernel_spmd

    if _nc_cache is None:
        _nc_cache = _build_program()
    nc = _nc_cache

    query = np.asarray(query, np.float32)
    key = np.asarray(key, np.float32)
    value = np.asarray(value, np.float32)
    mask = np.asarray(mask)
    Wq, bq = np.asarray(Wq, np.float32), np.asarray(bq, np.float32)
    Wk, bk = np.asarray(Wk, np.float32), np.asarray(bk, np.float32)
    Wv, bvv = np.asarray(Wv, np.float32), np.asarray(bv, np.float32)
    Wo, bo = np.asarray(Wo, np.float32), np.asarray(bo, np.float32)

    # host-side shard/layout prep
    xqT = [np.ascontiguousarray(query[b].T) for b in range(B)]
    xkT = [np.ascontiguousarray(key[b].T) for b in range(B)]
    xvT = [np.ascontiguousarray(value[b].T) for b in range(B)]
    m8 = [np.ascontiguousarray(mask[b].astype(np.int8)) for b in range(B)]

    in_maps = []
    for c in range(N_CORES):
        b = c // CPB
        h0 = (c % CPB) * HPC          # first head on this core
        lo, hi = h0 * DK, h0 * DK + HD
        in_maps.append({
            "xqT": xqT[b], "xkT": xkT[b], "xvT": xvT[b], "m8": m8[b],
            "wq": np.ascontiguousarray(Wq[:, lo:hi]),
            "wk": np.ascontiguousarray(Wk[:, lo:hi]),
            "wv": np.ascontiguousarray(Wv[:, lo:hi]),
            "wo": np.ascontiguousarray(
                np.concatenate([
                    np.concatenate([Wo[lo + c * DK:lo + (c + 1) * DK, :],
                                    (bo / 16.0).reshape(1, D)], axis=0)
                    for c in range(HPC)], axis=0)),
            "bqT": np.ascontiguousarray(bq[lo:hi].reshape(HD // P, P).T),
            "bkT": np.ascontiguousarray(bk[lo:hi].reshape(HD // P, P).T),
            "bv": np.ascontiguousarray(bvv[lo:hi].reshape(1, HD)),

        })

    res = run_bass_kernel_spmd(nc, in_maps, core_ids=list(range(N_CORES)),
                               trace=trace)

    attention = np.empty((B, H, S, S), np.float32)
    out = np.zeros((B, S, D), np.float32)
    for c in range(N_CORES):
        b = c // CPB
        h0 = (c % CPB) * HPC
        attention[b, h0:h0 + HPC] = res.results[c]["attn"]
        out[b] += res.results[c]["outp"]

    if trace:
        kernel.last_exec_time_ns = res.exec_time_ns
        kernel.last_results = res
    return (out, attention)
